# revision 1
# baseline (speedup 1.0000x reference)
"""GAT (2-layer, 8-head) Bass kernel for 8 Trainium2 NeuronCores.

Sharding: rows (nodes) split 512/core; x and params replicated.
Layer-1 attention per head computed in transposed layout e.T[j, i] =
lrelu(f1[i] + f2[j]); softmax row-sums fold into the aggregation matmul
via an augmented ones column.  Heads are split between two compute
forms to balance engines:
  A-form (ACT): e = Prelu(f1+f2); p = Exp(e); p *= mask; h.T = Wh_aug.T @ p
  B-form (DVE): branch masks M+/- = mask * [f1_i >= -f2_j] (exact {0,1}
    in bf16), h.T = E1*(X+.T @ M+) + G1*(X-.T @ M-) with X+/- = E2/G2-scaled
    Wh (separable exp: exp(lrelu(s)) = exp(s) or exp(0.2 s)).
Layer 2 needs full Wh2 = xc @ W_out -> small AllGather of per-core
[512, 41] shards.  elu's "-1" is algebraically absorbed (colsum
correction + log_softmax shift invariance).
"""
import sys

sys.path.insert(0, "/opt/trn_rl_repo")

import numpy as np
import ml_dtypes

import concourse.bass as bass
import concourse.bacc as bacc
import concourse.tile as tile
import concourse.mybir as mybir
from concourse.bass_utils import run_bass_kernel_spmd

F32 = mybir.dt.float32
BF16 = mybir.dt.bfloat16
AF = mybir.ActivationFunctionType
ALU = mybir.AluOpType
AX = mybir.AxisListType

NCORES = 8
N = 4096
FIN = 256
HID = 64
H = 8
NC = 41          # classes
ROWS = N // NCORES   # 512 rows per core
JT = N // 128        # 32 j tiles
IT = ROWS // 128     # 4 i tiles of my rows
AUG = HID + 1        # 65
AUG2 = NC + 1        # 42
ALPHA = 0.2

A_SET = (0,)                      # heads on ACT (exp/prelu) path
B_SET = (1, 2, 3, 4, 5, 6, 7)     # heads on DVE (branch-mask) path

_CACHED_NC = None


def _build(trace_sim=False, reps=1, ablate=()):
    nc = bacc.Bacc("TRN2", target_bir_lowering=False, debug=False,
                   num_devices=NCORES)
    d = {}
    def dram_in(name, shape, dt=F32):
        d[name] = nc.dram_tensor(name, list(shape), dt, kind="ExternalInput").ap()
        return d[name]

    xT = dram_in("xT", [128, 2, N])
    xrT = dram_in("xrT", [128, 2, ROWS])
    wcat = dram_in("wcat", [128, 2, H * HID])
    wa = dram_in("wa", [128, 2, 2 * H])
    wout = dram_in("wout", [128, 4, NC])
    woa1 = dram_in("woa1", [128, 4])
    a2b = dram_in("a2b", [128, NC])
    csum = dram_in("csum", [128, NC])
    ident = dram_in("ident", [128, NC])
    consts = dram_in("consts", [128, 8])
    maskT = dram_in("maskT", [128, JT, ROWS], BF16)
    out = nc.dram_tensor("out", [ROWS, NC], F32, kind="ExternalOutput").ap()

    with tile.TileContext(nc, trace_sim=trace_sim) as tc:
        with (
            tc.tile_pool(name="dram", bufs=1, space="DRAM") as dpool,
            tc.tile_pool(name="const", bufs=1) as cp,
            tc.tile_pool(name="big", bufs=1) as bigp,
            tc.tile_pool(name="work", bufs=3) as wp,
            tc.tile_pool(name="work2", bufs=2) as wp2,
            tc.tile_pool(name="head", bufs=2) as hp,
            tc.tile_pool(name="head1", bufs=1) as hp1,
            tc.tile_pool(name="psA", bufs=2, space="PSUM") as psA,
            tc.tile_pool(name="psB", bufs=2, space="PSUM") as psB,
            tc.tile_pool(name="psS", bufs=1, space="PSUM") as psS,
        ):
            # ---------------- stage 0: loads ----------------
            xrT_sb = cp.tile([128, 2, ROWS], F32)
            nc.sync.dma_start(out=xrT_sb[:], in_=xrT[:])
            wcat_sb = cp.tile([128, 2, H * HID], F32)
            nc.sync.dma_start(out=wcat_sb[:], in_=wcat[:])
            wa_sb = cp.tile([128, 2, 2 * H], F32)
            nc.sync.dma_start(out=wa_sb[:], in_=wa[:])
            wout_sb = cp.tile([128, 4, NC], F32)
            nc.sync.dma_start(out=wout_sb[:], in_=wout[:])
            woa1_sb = cp.tile([128, 4], F32)
            nc.sync.dma_start(out=woa1_sb[:], in_=woa1[:])
            a2b_sb = cp.tile([128, NC], F32)
            nc.sync.dma_start(out=a2b_sb[:], in_=a2b[:])
            csum_sb = cp.tile([128, NC], F32)
            nc.sync.dma_start(out=csum_sb[:], in_=csum[:])
            ident_sb = cp.tile([128, NC], F32)
            nc.sync.dma_start(out=ident_sb[:], in_=ident[:])
            consts_sb = cp.tile([128, 8], F32)
            nc.sync.dma_start(out=consts_sb[:], in_=consts[:])
            mask_sb = bigp.tile([128, JT, ROWS], BF16)
            nc.sync.dma_start(out=mask_sb[:], in_=maskT[:])

            def body():
                # ---------------- stage 1: Wh, F ----------------
                whaug = bigp.tile([128, JT, H, AUG], BF16)
                nc.gpsimd.memset(whaug[:, :, :, HID:AUG], 1.0)
                F_sb = cp.tile([128, JT, 2 * H], F32)
                for it in range(JT):
                    xt_t = wp.tile([128, 2, 128], F32, tag="xt")
                    nc.sync.dma_start(out=xt_t[:], in_=xT[:, :, it * 128:(it + 1) * 128])
                    pwh = psA.tile([128, H * HID], F32, tag="pa")
                    for kt in range(2):
                        nc.tensor.matmul(pwh[:], xt_t[:, kt, :],
                                         wcat_sb[:, kt, :], start=(kt == 0), stop=(kt == 1))
                    srcv = pwh.rearrange("p (h d) -> p h d", h=H)
                    dst = whaug[:, it, :, 0:HID]
                    nc.scalar.copy(dst, srcv)
                    pf = psS.tile([128, 2 * H], F32, tag="s")
                    for kt in range(2):
                        nc.tensor.matmul(pf[:], xt_t[:, kt, :],
                                         wa_sb[:, kt, :], start=(kt == 0), stop=(kt == 1))
                    nc.vector.tensor_copy(F_sb[:, it, :], pf[:])
                negF_bf = cp.tile([128, JT, 2 * H], BF16)
                nc.vector.tensor_scalar(negF_bf[:], F_sb[:], -1.0, None, op0=ALU.mult)
                E_sb = cp.tile([128, JT, 2 * H], BF16)
                nc.scalar.activation(E_sb[:], F_sb[:], AF.Exp)
                G_sb = cp.tile([128, JT, 2 * H], BF16)
                nc.scalar.activation(G_sb[:], F_sb[:], AF.Exp, scale=ALPHA)
                negG_sb = cp.tile([128, JT, 2 * H], BF16)
                nc.vector.tensor_scalar(negG_sb[:], G_sb[:], -1.0, None, op0=ALU.mult)

                # F for my rows, transposed: [16, 512]
                pfmy = psS.tile([2 * H, ROWS], F32, tag="s")
                for kt in range(2):
                    nc.tensor.matmul(pfmy[:], wa_sb[:, kt, :], xrT_sb[:, kt, :],
                                     start=(kt == 0), stop=(kt == 1))
                fmy_bf = cp.tile([2 * H, ROWS], BF16)
                nc.vector.tensor_copy(fmy_bf[:], pfmy[:])
                fmy_nbf = cp.tile([2 * H, ROWS], BF16)
                nc.vector.tensor_scalar(fmy_nbf[:], pfmy[:], -1.0, None, op0=ALU.mult)
                # R1 = G1/E1 = exp(-(1-alpha) f1); combine uses hs = pp + pm*R1
                # (E1 factor cancels in the normalize)
                Rmy = cp.tile([2 * H, ROWS], F32)
                nc.scalar.activation(Rmy[:], pfmy[:], AF.Exp, scale=-(1.0 - ALPHA))
                Rmy_neg = cp.tile([2 * H, ROWS], F32)
                nc.vector.tensor_scalar(Rmy_neg[:], Rmy[:], -1.0, None, op0=ALU.mult)

                xcT = bigp.tile([128, 4, ROWS], F32)

                # per-head normalize + elu' tail (writes xc+1; -1 absorbed later)
                def head_tail(hsrc, h):
                    if "tails" in ablate:
                        nc.vector.tensor_copy(
                            xcT[(h % 2) * HID:(h % 2) * HID + HID, h // 2, :],
                            hsrc[0:HID, :])
                        return
                    rr = hp1.tile([1, ROWS], F32, tag="rr")
                    nc.vector.reciprocal(rr[:], hsrc[HID:AUG, :])
                    rb = hp1.tile([128, ROWS], F32, tag="rb")
                    nc.gpsimd.partition_broadcast(rb[:], rr[:])
                    hn = hp1.tile([HID, ROWS], F32, tag="hn")
                    nc.vector.tensor_tensor(hn[:], hsrc[0:HID, :], rb[0:HID, :], op=ALU.mult)
                    tm = hp1.tile([HID, ROWS], F32, tag="tm")
                    nc.vector.tensor_scalar(tm[:], hn[:], 0.0, None, op0=ALU.min)
                    te = hp1.tile([HID, ROWS], F32, tag="te")
                    nc.scalar.activation(te[:], tm[:], AF.Exp)
                    dst = xcT[(h % 2) * HID:(h % 2) * HID + HID, h // 2, :]
                    nc.vector.scalar_tensor_tensor(dst, hn[:], 0.0, te[:],
                                                   op0=ALU.max, op1=ALU.add)

                # ---------------- stage 2: heads ----------------
                a_set, b_set = A_SET, B_SET
                if "A2" in ablate:
                    a_set, b_set = (0, 1), (2, 3, 4, 5, 6, 7)
                if "A4" in ablate:
                    a_set, b_set = (0, 1, 2, 3), (4, 5, 6, 7)
                if "A1" in ablate:
                    a_set, b_set = (0,), (1, 2, 3, 4, 5, 6, 7)
                if "A0" in ablate:
                    a_set, b_set = (), (0, 1, 2, 3, 4, 5, 6, 7)
                for h in a_set:
                    f1b = hp.tile([128, ROWS], BF16, tag="f1b")
                    f1s = hp1.tile([1, ROWS], BF16, tag="f1s")
                    nc.sync.dma_start(out=f1s[:], in_=fmy_bf[2 * h:2 * h + 1, :])
                    nc.gpsimd.partition_broadcast(f1b[:], f1s[:])
                    pa = psA.tile([AUG, ROWS], F32, tag="pa")
                    for g in range(JT // 4):
                        j0 = g * 4
                        if "A_act" in ablate:
                            for q in range(4):
                                jt = j0 + q
                                nc.tensor.matmul(pa[:], whaug[:, jt, h, :],
                                                 mask_sb[:, jt, :],
                                                 start=(jt == 0), stop=(jt == JT - 1))
                            continue
                        pt4 = wp2.tile([128, 4, ROWS], BF16, tag="pt")
                        for q in range(4):
                            jt = j0 + q
                            et = wp.tile([128, ROWS], F32, tag="et")
                            nc.scalar.activation(et[:], f1b[:], AF.Prelu,
                                                 bias=F_sb[:, jt, 2 * h + 1:2 * h + 2], alpha=ALPHA)
                            nc.scalar.activation(pt4[:, q, :], et[:], AF.Exp)
                        pmt = wp2.tile([128, 4, ROWS], BF16, tag="pmt")
                        nc.gpsimd.tensor_tensor(pmt[:], pt4[:], mask_sb[:, j0:j0 + 4, :],
                                                op=ALU.mult)
                        for q in range(4):
                            jt = j0 + q
                            nc.tensor.matmul(pa[:], whaug[:, jt, h, :], pmt[:, q, :],
                                             start=(jt == 0), stop=(jt == JT - 1))
                    head_tail(pa, h)

                for h in b_set:
                    f1b = hp.tile([128, ROWS], BF16, tag="f1b")
                    f1s = hp1.tile([1, ROWS], BF16, tag="f1s")
                    nc.sync.dma_start(out=f1s[:], in_=fmy_bf[2 * h:2 * h + 1, :])
                    nc.gpsimd.partition_broadcast(f1b[:], f1s[:])
                    Xp = hp.tile([128, JT, AUG], BF16, tag="Xp")
                    Xm = hp.tile([128, JT, AUG], BF16, tag="Xm")
                    nc.vector.tensor_tensor(Xp[:], whaug[:, :, h, :],
                        E_sb[:, :, 2 * h + 1:2 * h + 2].broadcast_to([128, JT, AUG]),
                        op=ALU.mult)
                    nc.vector.tensor_tensor(Xm[:], whaug[:, :, h, :],
                        G_sb[:, :, 2 * h + 1:2 * h + 2].broadcast_to([128, JT, AUG]),
                        op=ALU.mult)
                    Xmn = hp.tile([128, JT, AUG], BF16, tag="Xmn")
                    if "C3" not in ablate:
                        nc.vector.tensor_tensor(Xmn[:], whaug[:, :, h, :],
                            negG_sb[:, :, 2 * h + 1:2 * h + 2].broadcast_to([128, JT, AUG]),
                            op=ALU.mult)
                    pp = psB.tile([AUG, ROWS], F32, tag="pp")
                    pm = psB.tile([AUG, ROWS], F32, tag="pm")
                    f1b3 = f1b[:].rearrange("p (o r) -> p o r", o=1)
                    if "ST1" in ablate:
                        f1nb = hp.tile([128, ROWS], BF16, tag="f1nb")
                        f1ns = hp1.tile([1, ROWS], BF16, tag="f1s")
                        nc.sync.dma_start(out=f1ns[:], in_=fmy_nbf[2 * h:2 * h + 1, :])
                        nc.gpsimd.partition_broadcast(f1nb[:], f1ns[:])
                        for jt in range(JT):
                            mpt1 = wp.tile([128, ROWS], BF16, tag="mp1")
                            nc.vector.scalar_tensor_tensor(mpt1[:], f1nb[:],
                                F_sb[:, jt, 2 * h + 1:2 * h + 2], mask_sb[:, jt, :],
                                op0=ALU.is_le, op1=ALU.mult)
                            nc.tensor.matmul(pp[:], Xp[:, jt, :], mpt1[:],
                                             start=(jt == 0), stop=(jt == JT - 1))
                            nc.tensor.matmul(pm[:], Xmn[:, jt, :], mask_sb[:, jt, :],
                                             start=(jt == 0), stop=False)
                            nc.tensor.matmul(pm[:], Xm[:, jt, :], mpt1[:],
                                             start=False, stop=(jt == JT - 1))
                    for g in range(0 if "ST1" in ablate else JT // 4):
                        j0 = g * 4
                        mgrp = mask_sb[:, j0:j0 + 4, :]
                        if "B_mask" in ablate:
                            for q in range(4):
                                jt = j0 + q
                                nc.tensor.matmul(pp[:], Xp[:, jt, :], mask_sb[:, jt, :],
                                                 start=(jt == 0), stop=(jt == JT - 1))
                                if "B_mm" not in ablate:
                                    nc.tensor.matmul(pm[:], Xm[:, jt, :], mask_sb[:, jt, :],
                                                     start=(jt == 0), stop=(jt == JT - 1))
                            continue
                        prt = wp2.tile([128, 4, ROWS], BF16, tag="prt")
                        nc.vector.tensor_tensor(prt[:],
                            f1b3.broadcast_to([128, 4, ROWS]),
                            negF_bf[:, j0:j0 + 4, 2 * h + 1:2 * h + 2]
                                .broadcast_to([128, 4, ROWS]),
                            op=ALU.is_ge)
                        mpt = wp2.tile([128, 4, ROWS], BF16, tag="mpt")
                        nc.vector.tensor_tensor(mpt[:], prt[:], mgrp, op=ALU.mult)
                        if "C3" in ablate:
                            mmt = wp2.tile([128, 4, ROWS], BF16, tag="mmt")
                            nc.vector.tensor_tensor(mmt[:], mgrp, mpt[:], op=ALU.subtract)
                            for q in range(4):
                                jt = j0 + q
                                nc.tensor.matmul(pp[:], Xp[:, jt, :], mpt[:, q, :],
                                                 start=(jt == 0), stop=(jt == JT - 1))
                                nc.tensor.matmul(pm[:], Xm[:, jt, :], mmt[:, q, :],
                                                 start=(jt == 0), stop=(jt == JT - 1))
                        else:
                            # pm' = (-Xm)@mask + Xm@mpt = -(Xm@mmt); sign folded
                            # into the negated R1 at combine time
                            for q in range(4):
                                jt = j0 + q
                                nc.tensor.matmul(pp[:], Xp[:, jt, :], mpt[:, q, :],
                                                 start=(jt == 0), stop=(jt == JT - 1))
                                nc.tensor.matmul(pm[:], Xmn[:, jt, :], mask_sb[:, jt, :],
                                                 start=(jt == 0), stop=False)
                                nc.tensor.matmul(pm[:], Xm[:, jt, :], mpt[:, q, :],
                                                 start=False, stop=(jt == JT - 1))
                    r1s = hp1.tile([1, ROWS], F32, tag="r1s")
                    rsrc = Rmy if "C3" in ablate else Rmy_neg
                    nc.sync.dma_start(out=r1s[:], in_=rsrc[2 * h:2 * h + 1, :])
                    r1b = hp1.tile([128, ROWS], F32, tag="r1b")
                    nc.gpsimd.partition_broadcast(r1b[:], r1s[:])
                    t2 = hp1.tile([AUG, ROWS], F32, tag="t2")
                    src_pm = pp if "B_mm" in ablate else pm
                    nc.vector.tensor_tensor(t2[:], src_pm[:], r1b[0:AUG, :], op=ALU.mult)
                    hs = hp1.tile([AUG, ROWS], F32, tag="hs")
                    nc.vector.tensor_tensor(hs[:], pp[:], t2[:], op=ALU.add)
                    head_tail(hs, h)

                # ---------------- stage 3: Wh2 + gather ----------------
                wh2_sb = cp.tile([128, 4, NC], F32)
                for it in range(IT):
                    pw2 = psS.tile([128, NC], F32, tag="s")
                    for kt in range(4):
                        nc.tensor.matmul(pw2[:], xcT[:, kt, it * 128:(it + 1) * 128],
                                         wout_sb[:, kt, :], start=(kt == 0), stop=(kt == 3))
                    nc.vector.scalar_tensor_tensor(wh2_sb[:, it, :], pw2[:], 0.0,
                                                   csum_sb[:], op0=ALU.add, op1=ALU.subtract)
                ag_in = dpool.tile([128, 4, NC], F32)
                nc.gpsimd.dma_start(ag_in[:], wh2_sb[:])
                ag_out = dpool.tile([NCORES, 128, 4, NC], F32)
                if "coll" in ablate:
                    for r in range(NCORES):
                        nc.gpsimd.dma_start(ag_out[r], ag_in[:])
                else:
                    nc.gpsimd.collective_compute(
                        "AllGather", ALU.bypass,
                        replica_groups=[list(range(NCORES))],
                        ins=[ag_in.opt()], outs=[ag_out.opt()],
                    )
                # gathered rows: core r, it, p -> global row r*512 + it*128 + p
                # j-tile jt = (r*512 + it*128 + p)//128 ... partition p stays, tile idx = r*4+it
                wh2f = cp.tile([128, JT, AUG2], F32)
                nc.gpsimd.memset(wh2f[:, :, NC:AUG2], 1.0)
                for r in range(NCORES):
                    nc.sync.dma_start(out=wh2f[:, r * 4:(r + 1) * 4, 0:NC],
                                      in_=ag_out[r])

                # f1 for my rows (layer 2): [1, 512] psum
                pf1o = psS.tile([1, ROWS], F32, tag="s")
                for kt in range(4):
                    nc.tensor.matmul(pf1o[:], woa1_sb[:, kt:kt + 1],
                                     xcT[:, kt, :], start=(kt == 0), stop=(kt == 3))
                R1o = cp.tile([1, ROWS], F32)
                nc.scalar.activation(R1o[:], pf1o[:], AF.Exp, scale=-(1.0 - ALPHA),
                                     bias=consts_sb[0:1, 2:3])
                R1o_neg = cp.tile([1, ROWS], F32)
                nc.vector.tensor_scalar(R1o_neg[:], R1o[:], -1.0, None, op0=ALU.mult)
                f1o_bf = cp.tile([1, ROWS], BF16)
                nc.scalar.activation(f1o_bf[:], pf1o[:], AF.Identity, bias=consts_sb[0:1, 0:1])

                # f2 for all nodes (layer 2)
                f2o = cp.tile([128, JT], F32)
                t41b = hp1.tile([128, JT, NC], F32, tag="t41b")
                a2b3 = a2b_sb[:].rearrange("p (o c) -> p o c", o=1)
                nc.vector.tensor_tensor(t41b[:], wh2f[:, :, 0:NC],
                    a2b3.broadcast_to([128, JT, NC]), op=ALU.mult)
                nc.vector.reduce_sum(f2o[:].rearrange("p (k o) -> p k o", o=1),
                                     t41b[:], axis=AX.X)
                negf2o = cp.tile([128, JT], F32)
                nc.vector.tensor_scalar(negf2o[:], f2o[:], -1.0, None, op0=ALU.mult)
                E2o = cp.tile([128, JT], F32)
                nc.scalar.activation(E2o[:], f2o[:], AF.Exp)
                G2o = cp.tile([128, JT], F32)
                nc.scalar.activation(G2o[:], f2o[:], AF.Exp, scale=ALPHA)
                negG2o = cp.tile([128, JT], F32)
                nc.vector.tensor_scalar(negG2o[:], G2o[:], -1.0, None, op0=ALU.mult)

                # ---------------- layer-2 attention (B-form) ----------------
                f1b2 = hp.tile([128, ROWS], BF16, tag="f1b")
                nc.gpsimd.partition_broadcast(f1b2[:], f1o_bf[:])
                Xp2 = hp.tile([128, JT, AUG2], BF16, tag="Xp")
                Xm2 = hp.tile([128, JT, AUG2], BF16, tag="Xm")
                E2o3 = E2o[:].rearrange("p (k o) -> p k o", o=1)
                G2o3 = G2o[:].rearrange("p (k o) -> p k o", o=1)
                nc.vector.tensor_tensor(Xp2[:], wh2f[:],
                    E2o3.broadcast_to([128, JT, AUG2]), op=ALU.mult)
                nc.vector.tensor_tensor(Xm2[:], wh2f[:],
                    G2o3.broadcast_to([128, JT, AUG2]), op=ALU.mult)
                if "C3" not in ablate:
                    Xm2n = hp.tile([128, JT, AUG2], BF16, tag="Xmn")
                    negG2o3 = negG2o[:].rearrange("p (k o) -> p k o", o=1)
                    nc.vector.tensor_tensor(Xm2n[:], wh2f[:],
                        negG2o3.broadcast_to([128, JT, AUG2]), op=ALU.mult)
                pp2 = psB.tile([AUG2, ROWS], F32, tag="pp")
                pm2 = psB.tile([AUG2, ROWS], F32, tag="pm")
                negf2o_bf = cp.tile([128, JT], BF16)
                nc.vector.tensor_copy(negf2o_bf[:], negf2o[:])
                f1b23 = f1b2[:].rearrange("p (o r) -> p o r", o=1)
                for g in range(JT // 4):
                    j0 = g * 4
                    mgrp = mask_sb[:, j0:j0 + 4, :]
                    prt = wp2.tile([128, 4, ROWS], BF16, tag="prt")
                    nc.vector.tensor_tensor(prt[:],
                        f1b23.broadcast_to([128, 4, ROWS]),
                        negf2o_bf[:, j0:j0 + 4].rearrange("p (k o) -> p k o", o=1)
                            .broadcast_to([128, 4, ROWS]),
                        op=ALU.is_ge)
                    mpt = wp2.tile([128, 4, ROWS], BF16, tag="mpt")
                    nc.vector.tensor_tensor(mpt[:], prt[:], mgrp, op=ALU.mult)
                    if "C3" not in ablate:
                        for q in range(4):
                            jt = j0 + q
                            nc.tensor.matmul(pp2[:], Xp2[:, jt, :], mpt[:, q, :],
                                             start=(jt == 0), stop=(jt == JT - 1))
                            nc.tensor.matmul(pm2[:], Xm2n[:, jt, :], mask_sb[:, jt, :],
                                             start=(jt == 0), stop=False)
                            nc.tensor.matmul(pm2[:], Xm2[:, jt, :], mpt[:, q, :],
                                             start=False, stop=(jt == JT - 1))
                    else:
                        mmt = wp2.tile([128, 4, ROWS], BF16, tag="mmt")
                        nc.vector.tensor_tensor(mmt[:], mgrp, mpt[:], op=ALU.subtract)
                        for q in range(4):
                            jt = j0 + q
                            nc.tensor.matmul(pp2[:], Xp2[:, jt, :], mpt[:, q, :],
                                             start=(jt == 0), stop=(jt == JT - 1))
                            nc.tensor.matmul(pm2[:], Xm2[:, jt, :], mmt[:, q, :],
                                             start=(jt == 0), stop=(jt == JT - 1))
                r1b2 = hp1.tile([128, ROWS], F32, tag="r1b")
                nc.gpsimd.partition_broadcast(r1b2[:], R1o[:] if "C3" in ablate else R1o_neg[:])
                t2 = hp1.tile([AUG2, ROWS], F32, tag="t2")
                nc.vector.tensor_tensor(t2[:], pm2[:], r1b2[0:AUG2, :], op=ALU.mult)
                hs2 = hp1.tile([AUG2, ROWS], F32, tag="hs")
                nc.vector.tensor_tensor(hs2[:], pp2[:], t2[:], op=ALU.add)
                # normalize + elu'
                srow2 = hp1.tile([1, ROWS], F32, tag="r1s")
                nc.sync.dma_start(out=srow2[:], in_=hs2[NC:AUG2, :])
                rr2 = hp1.tile([1, ROWS], F32, tag="rr")
                nc.vector.reciprocal(rr2[:], srow2[:])
                rb2 = hp1.tile([128, ROWS], F32, tag="rb")
                nc.gpsimd.partition_broadcast(rb2[:], rr2[:])
                zn = hp1.tile([NC, ROWS], F32, tag="hn")
                nc.vector.tensor_tensor(zn[:], hs2[0:NC, :], rb2[0:NC, :], op=ALU.mult)
                tm2 = hp1.tile([NC, ROWS], F32, tag="tm")
                nc.vector.tensor_scalar(tm2[:], zn[:], 0.0, None, op0=ALU.min)
                te2 = hp1.tile([NC, ROWS], F32, tag="te")
                nc.scalar.activation(te2[:], tm2[:], AF.Exp)
                zel = hp1.tile([NC, ROWS], F32, tag="zel")
                nc.vector.scalar_tensor_tensor(zel[:], zn[:], 0.0, te2[:],
                                               op0=ALU.max, op1=ALU.add)

                # ---------------- stage 4: log_softmax + out ----------------
                outr = out.rearrange("(t p) c -> p t c", p=128)
                for it in range(IT):
                    ztp = psS.tile([128, NC], F32, tag="s")
                    nc.tensor.transpose(ztp[:], zel[:, it * 128:(it + 1) * 128],
                                        ident_sb[0:NC, 0:NC])
                    zmax = wp.tile([128, 1], F32, tag="zmax")
                    nc.vector.reduce_max(zmax[:], ztp[:], axis=AX.X)
                    nzmax = wp.tile([128, 1], F32, tag="nzmax")
                    nc.vector.tensor_scalar(nzmax[:], zmax[:], -1.0, None, op0=ALU.mult)
                    zsum = wp.tile([128, 1], F32, tag="zsum")
                    zs = wp.tile([128, NC], F32, tag="zs")
                    nc.scalar.activation(zs[:], ztp[:], AF.Exp, bias=nzmax[:],
                                         accum_out=zsum[:])
                    lse = wp.tile([128, 1], F32, tag="lse")
                    nc.scalar.activation(lse[:], zsum[:], AF.Ln)
                    bo = wp.tile([128, 1], F32, tag="bo")
                    nc.vector.scalar_tensor_tensor(bo[:], zmax[:], -1.0, lse[:],
                                                   op0=ALU.mult, op1=ALU.subtract)
                    zf = wp.tile([128, NC], F32, tag="zf")
                    nc.scalar.activation(zf[:], ztp[:], AF.Identity, bias=bo[:])
                    nc.sync.dma_start(out=outr[:, it, :], in_=zf[:])

            for _rep in range(reps):
                body()

    nc.compile()
    return nc


def _host_prep(x, adj, W, a, W_out, a_out):
    bf16 = ml_dtypes.bfloat16
    f32 = np.float32
    x = np.asarray(x, f32)
    W = np.asarray(W, f32)
    a = np.asarray(a, f32)
    W_out = np.asarray(W_out, f32)
    a_out = np.asarray(a_out, f32)

    def pk(arr, kt):  # [kt*128, M] -> [128, kt, M]
        return np.ascontiguousarray(
            arr.reshape(kt, 128, *arr.shape[1:]).transpose(1, 0, *range(2, arr.ndim + 1)))

    xT = pk(np.ascontiguousarray(x.T), 2)                      # [128,2,4096]
    wcat = pk(np.concatenate(list(W), axis=1), 2)              # [128,2,512]
    WA = np.zeros((FIN, 2 * H), f32)
    for h in range(H):
        WA[:, 2 * h] = W[h] @ a[h, :HID]
        WA[:, 2 * h + 1] = W[h] @ a[h, HID:]
    wa = pk(WA, 2)
    wout = pk(W_out, 4)                                        # [128,4,41]
    Woa1 = W_out @ a_out[:NC]                                  # [512]
    woa1 = np.ascontiguousarray(Woa1.reshape(4, 128).T)        # [128,4]
    s = float(Woa1.sum())
    a2b = np.ascontiguousarray(np.broadcast_to(a_out[NC:], (128, NC)))
    csum = np.ascontiguousarray(np.broadcast_to(W_out.sum(0), (128, NC)))
    ident = np.eye(128, NC, dtype=f32)
    consts = np.zeros((128, 8), f32)
    consts[:, 0] = -s
    consts[:, 1] = -ALPHA * s
    consts[:, 2] = (1.0 - ALPHA) * s

    shared = dict(xT=xT, wcat=wcat, wa=wa, wout=wout, woa1=woa1, a2b=a2b,
                  csum=csum, ident=ident, consts=consts)
    in_maps = []
    for c in range(NCORES):
        rows = slice(c * ROWS, (c + 1) * ROWS)
        mT = (np.asarray(adj[rows]).T > 0).astype(bf16)        # [4096, 512]
        mT = np.ascontiguousarray(mT.reshape(JT, 128, ROWS).transpose(1, 0, 2))
        xr = pk(np.ascontiguousarray(x[rows].T), 2)            # [128,2,512]
        in_maps.append({**shared, "maskT": mT, "xrT": xr})
    return in_maps


def kernel(x, adj, W, a, W_out, a_out):
    global _CACHED_NC
    if _CACHED_NC is None:
        _CACHED_NC = _build()
    in_maps = _host_prep(x, adj, W, a, W_out, a_out)
    res = run_bass_kernel_spmd(_CACHED_NC, in_maps, list(range(NCORES)))
    out = np.concatenate([res.results[c]["out"] for c in range(NCORES)], axis=0)
    return out.astype(np.float32)

